# revision 56
# baseline (speedup 1.0000x reference)
"""Trainium2 Bass kernel for nn_ContextLabel (GNN label propagation).

Computation: 10 iterations of Y = masked(adj @ Y) on [10000,16], then
straight-through gumbel one-hot, dist = (adj!=0) @ Yh row-normalized,
output mean((dist - pseudo_labels)^2)  (scalar).

The per-step update is affine: Y <- B @ Y + c with B = diag(1-m) adj
and c = m*labels, so two steps fuse into one pass with the squared
operator: Y <- B^2 @ Y + (Bc + c).  The device runs 5 fused passes
(temporal blocking) instead of 10 - identical HBM traffic (one operator
matrix + one mask matrix streamed per core, both fp8), but half the
tensor-engine passes and half the AllGathers.  B^2 is ~810 nnz/row and
is scaled by 64 before fp8 quantization (entries ~1e-3 would underflow
e4m3 otherwise); the on-chip Y state runs at 64x scale (c2 and gumbel
prescaled on host, argmax is scale-invariant) and the exact 1/64
descale rides the fp8 cast feeding each AllGather, so it costs nothing.

Strategy (8 NeuronCores, row-parallel, padded to 1280 rows/core):
 - core c owns rows [1250c, 1250c+1250), zero-padded to 1280 so the
   padded N is 10240 = 80 chunks of 128 exactly.
 - (64*B^2)^T shard (fp8 e4m3, [10240 x 1280]) stays RESIDENT in SBUF;
   all 5 passes stream it from SBUF through the tensor engine with
   Y stationary: out^T[16,1280] = Y^T @ opT, split over 4 PE column
   groups (tile_position at partitions 0/32/64/96) so the four matmuls
   of each contraction chunk stream concurrently (~3x PE throughput).
 - per-iteration AllGather of the fp8 Y slice in chunk-tiled p-major
   layout [128,10,16] so both collective-side DMAs are clean line
   transfers; operator loads are spread over all 3 DMA-capable queues
   (scalar/gpsimd/sync, ~100GB/s each) and mask prefetches are
   dependency-gated into pass windows (2 per window, 2 queues) so
   nothing contends with the latency-bound collectives.
 - COMPACT PASS 0: the initial Y is c = mask*labels, zero outside the
   ~1000 train rows, so pass 0 contracts only the packed train columns
   of B^2 (exact, ~1.9MB vs 12.5MB).  Every core reaches the first
   AllGather ~8us after ITS start instead of after the full operator
   load, so cross-core startup skew overlaps the background load of
   the full operator (used from pass 1 on).
 - gumbel straight-through one-hot computed LOCALLY on the core's own
   rows; the final exchange gathers the fp8 one-hot (exact in fp8).
 - final pass streams the 0/1 mask of the ORIGINAL adj (fp8): 8/10
   groups prefetched into SBUF during iteration pass windows, the last
   2 loaded into the freed operator pool slots during pass 5.
Verified on host: 1 argmax flip out of 10000 rows, final relerr ~4e-6
(tolerance 2e-2).
"""

import hashlib
import os
import shutil
import sys
from pathlib import Path

import numpy as np
import ml_dtypes

sys.path.insert(0, "/opt/trn_rl_repo")

import concourse.bass as bass  # noqa: E402
import concourse.mybir as mybir  # noqa: E402
import concourse.tile as tile  # noqa: E402
from concourse import bacc  # noqa: E402
import concourse.bass2jax as bass2jax  # noqa: E402
from concourse.bass_utils import run_bass_kernel_spmd  # noqa: E402

F8 = ml_dtypes.float8_e4m3
NCORES = 8
N = 10000
C = 16
R = N // NCORES           # 1250 real rows per core
RP = 1280                 # padded rows per core
NP = RP * NCORES          # 10240 padded N
NB = RP // 128            # 10 local blocks of 128 rows
NCH = NP // 128           # 80 contraction chunks of 128
NG = NCH // 8             # 10 adjT groups of 8 chunks
# PE column-group strips: (partition base, col offset, width)
STRIPS = [(0, 0, 384), (32, 384, 384), (64, 768, 256), (96, 1024, 256)]
# local block b -> (strip partition base, col offset within strip)
BLK = [(0, 0), (0, 128), (0, 256), (32, 0), (32, 128), (32, 256),
       (64, 0), (64, 128), (96, 0), (96, 128)]
MRES = 8                  # mask groups resident in SBUF
NITER = 5                 # fused 2-step passes (10 reference steps)
SCALE = 64.0              # fp8 scale for B^2 (descaled in the AG cast)
DR = False                # fp8 DoubleRow matmuls (K=256 per instruction)

_NEFF_CACHE = Path.home() / ".cache" / "bass_neff"


def _install_neff_cache():
    orig = bass2jax.compile_bir_kernel
    if getattr(bass2jax.compile_bir_kernel, "_cached", False):
        return

    def cached(bir_json, tmpdir, neff_name="file.neff"):
        h = hashlib.sha256(bir_json).hexdigest()
        p = _NEFF_CACHE / f"{h}.neff"
        dst = os.path.join(tmpdir, neff_name)
        if p.exists():
            shutil.copy(p, dst)
            return dst
        out = orig(bir_json, tmpdir, neff_name)
        try:
            _NEFF_CACHE.mkdir(parents=True, exist_ok=True)
            shutil.copy(out, p)
        except OSError:
            pass
        return out

    cached._cached = True
    bass2jax.compile_bir_kernel = cached


def build_program(tcch=12):
    nc = bacc.Bacc(
        "TRN2", target_bir_lowering=False, debug=False,
        enable_asserts=False, num_devices=NCORES,
    )
    f8, f16, f32 = mybir.dt.float8e4, mybir.dt.float16, mybir.dt.float32
    u8 = mybir.dt.uint8

    # pre-tiled p-major [128, chunk, col] so group loads are contiguous
    adjT_d = nc.dram_tensor("adjT8", [128, NCH * RP], f8, kind="ExternalInput")
    maskT_d = nc.dram_tensor("maskT8", [128, NCH * RP], f8, kind="ExternalInput")
    # compact pass-0 operand: B^2 restricted to the train columns
    # (c is zero elsewhere, so this is exact), tcch chunks of 128
    btc_d = nc.dram_tensor("b2tc8", [128, tcch * RP], f8,
                           kind="ExternalInput")
    y0c_d = nc.dram_tensor("y0c", [128, tcch * C], f8, kind="ExternalInput")
    guml_d = nc.dram_tensor("gumloc", [128, NB * C], f32, kind="ExternalInput")
    c2_d = nc.dram_tensor("c2loc", [128, NB * C], f16, kind="ExternalInput")
    lloc_d = nc.dram_tensor("lloc", [128, NB * C], f16, kind="ExternalInput")
    mloc_d = nc.dram_tensor("mloc", [128, NB * C], u8, kind="ExternalInput")
    pst_d = nc.dram_tensor("pst", [128, NB * C], f32, kind="ExternalInput")
    id16_d = nc.dram_tensor("id416", [128, C], f16, kind="ExternalInput")
    id32_d = nc.dram_tensor("id432", [128, C], f32, kind="ExternalInput")
    out_d = nc.dram_tensor("out_sq", [128, NB], f32, kind="ExternalOutput")
    DBG = False
    if DBG:
        dbg_y1_d = nc.dram_tensor("dbg_y1", [128, NB * C], f16,
                                  kind="ExternalOutput")
        dbg_yc_d = nc.dram_tensor("dbg_yc", [128, NCH * C], f8,
                                  kind="ExternalOutput")
        dbg_y5_d = nc.dram_tensor("dbg_y5", [128, NB * C], f16,
                                  kind="ExternalOutput")
        dbg_yh_d = nc.dram_tensor("dbg_yh", [128, NB * C], f16,
                                  kind="ExternalOutput")
        dbg_dist_d = nc.dram_tensor("dbg_dist", [128, NB * C], f32,
                                    kind="ExternalOutput")

    with tile.TileContext(nc) as tc:
        with (
            tc.tile_pool(name="sb", bufs=1) as sb,
            tc.tile_pool(name="ps", bufs=2, space="PSUM") as ps,
            tc.tile_pool(name="dram", bufs=2, space="DRAM") as dram,
        ):
            # ---- resident tiles -------------------------------------
            at_g = [sb.tile([128, 8 * RP], f8, name=f"at{g}", tag=f"at{g}")
                    for g in range(NG)]
            mt_res = [sb.tile([128, 8 * RP], f8, name=f"mt{g}", tag=f"mt{g}")
                      for g in range(MRES)]
            ycur = sb.tile([128, NCH * C], f8)
            btc = sb.tile([128, tcch * RP], f8)
            y0c = sb.tile([128, tcch * C], f8)
            yT = sb.tile([128, 384], f16)
            yloc = sb.tile([128, NB * C], f16)
            yloc8 = sb.tile([128, NB * C], f8)
            gumloc = sb.tile([128, NB * C], f32)
            c2loc = sb.tile([128, NB * C], f16)
            lloc = sb.tile([128, NB * C], f16)
            mloc = sb.tile([128, NB * C], u8)
            pst = sb.tile([128, NB * C], f32)
            id16 = sb.tile([128, C], f16)
            id32 = sb.tile([128, C], f32)

            # ---- initial loads --------------------------------------
            # small tensors first (sync queue); adjT group loads on two
            # queues, even/odd interleaved so arrival tracks consumption
            # compact pass-0 operands first (small, 3 queues) so every
            # core reaches the first AllGather fast regardless of skew
            nc.sync.dma_start(out=y0c[:], in_=y0c_d[:])
            # whole compact operand on the sync queue: the pass-0
            # collective's ccin also lives there, so nothing big may
            # precede it (the full operator loads stay off sync)
            nc.sync.dma_start(out=btc[:], in_=btc_d[:])
            nc.sync.dma_start(out=id16[:], in_=id16_d[:])
            nc.sync.dma_start(out=c2loc[:], in_=c2_d[:])
            nc.sync.dma_start(out=lloc[:], in_=lloc_d[:])
            nc.sync.dma_start(out=mloc[:], in_=mloc_d[:])
            nc.sync.dma_start(out=gumloc[:], in_=guml_d[:])
            nc.sync.dma_start(out=pst[:], in_=pst_d[:])
            nc.sync.dma_start(out=id32[:], in_=id32_d[:])
            # full operator loads on scalar+gpsimd only (5 groups each,
            # ~65us wall) - pass 1 cannot start before ~50us anyway, and
            # keeping them OFF the sync queue lets pass-0's collective
            # DMAs run immediately after the compact pass
            load_engs = [nc.scalar, nc.gpsimd]
            for g in range(NG):
                load_engs[g % 2].dma_start(
                    out=at_g[g][:],
                    in_=adjT_d[:, g * 8 * RP:(g + 1) * 8 * RP],
                )

            def mm_pass0(acc):
                """compact pass 0: contract only the train columns."""
                for k in range(tcch):
                    lhsT = y0c[:, k * C:(k + 1) * C]
                    for (pb, co, w) in STRIPS:
                        nc.tensor.matmul(
                            acc[pb:pb + C, 0:w],
                            lhsT,
                            btc[:, k * RP + co:k * RP + co + w],
                            start=(k == 0), stop=(k == tcch - 1),
                            tile_position=(0, pb),
                        )

            def mm_pass(acc, lhs_tile, chunk_tile):
                """acc strips += lhs.T @ adjT over all 80 chunks.

                Chunk-major so the 4 column-group matmuls of each chunk
                stream concurrently through the PE array.
                """
                if DR:
                    for k in range(0, NCH, 2):
                        g, j = divmod(k, 8)
                        lhsT = lhs_tile[:, k * C:(k + 2) * C].rearrange(
                            "p (two c) -> p c two", two=2)
                        rt3 = chunk_tile(g)[:].rearrange(
                            "p (j x) -> p x j", j=8)
                        for (pb, co, w) in STRIPS:
                            nc.tensor.matmul(
                                acc[pb:pb + C, 0:w],
                                lhsT,
                                rt3[:, co:co + w, j:j + 2],
                                start=(k == 0), stop=(k == NCH - 2),
                                tile_position=(0, pb),
                                perf_mode=mybir.MatmulPerfMode.DoubleRow,
                            )
                    return
                for k in range(NCH):
                    g, j = divmod(k, 8)
                    lhsT = lhs_tile[:, k * C:(k + 1) * C]
                    rt = chunk_tile(g)
                    for (pb, co, w) in STRIPS:
                        nc.tensor.matmul(
                            acc[pb:pb + C, 0:w],
                            lhsT,
                            rt[:, j * RP + co:j * RP + co + w],
                            start=(k == 0), stop=(k == NCH - 1),
                            tile_position=(0, pb),
                        )

            def strip_copies(dst, acc):
                """psum strips -> sbuf, alternating scalar/vector engines."""
                for i, (pb, co, w) in enumerate(STRIPS):
                    if i % 2 == 0:
                        nc.scalar.copy(dst[pb:pb + C, 0:w], acc[pb:pb + C, 0:w])
                    else:
                        nc.vector.tensor_copy(dst[pb:pb + C, 0:w],
                                              acc[pb:pb + C, 0:w])

            def transposes(trp, src, ident):
                for b in range(NB):
                    pb, o = BLK[b]
                    nc.tensor.transpose(
                        trp[:, b * C:(b + 1) * C],
                        src[pb:pb + C, o:o + 128],
                        ident[pb:pb + C, :],
                        tile_position=(pb, 0),
                    )

            # ---- 5 fused 2-step propagation passes ------------------
            mt_s = []
            for t in range(NITER):
                acc = ps.tile([128, 384], f32, name=f"acc{t}", tag="acc")
                if t == 0:
                    mm_pass0(acc)
                else:
                    mm_pass(acc, ycur, lambda g: at_g[g])
                strip_copies(yT, acc)
                trp = ps.tile([128, NB * C], f16, name=f"trp{t}", tag="trp")
                transposes(trp, yT, id16)
                # 64Y <- (64 B^2) Y + 64(Bc + c)
                nc.vector.tensor_tensor(yloc[:], trp[:], c2loc[:],
                                        mybir.AluOpType.add)
                if DBG and t == 0:
                    nc.sync.dma_start(out=dbg_y1_d[:], in_=yloc[:])
                if DBG and t == NITER - 1:
                    nc.sync.dma_start(out=dbg_y5_d[:], in_=yloc[:])
                if t < NITER - 1:
                    # yloc carries 64*Y; descale exactly in the fp8 cast
                    nc.vector.tensor_scalar_mul(yloc8[:], yloc[:], 1.0 / SCALE)
                    cc_in = dram.tile([128, NB * C], f8, name=f"ccin{t}",
                                      tag="ccin")
                    cc_out = dram.tile([NCORES * 128, NB * C], f8,
                                       name=f"ccout{t}", tag="ccout",
                                       addr_space="Shared")
                    nc.sync.dma_start(out=cc_in[:], in_=yloc8[:])
                    nc.gpsimd.collective_compute(
                        "AllGather", mybir.AluOpType.bypass,
                        replica_groups=[list(range(NCORES))],
                        ins=[cc_in[:]], outs=[cc_out[:]],
                    )
                    nc.sync.dma_start(
                        out=ycur[:].rearrange("p (g x) -> p g x", g=NCORES),
                        in_=cc_out[:].rearrange("(g p) x -> p g x", p=128),
                    )
                    # mask prefetch, gated on the post-collective gather
                    # (tiny write into the target tile) so the load runs
                    # during the NEXT pass, when HBM is otherwise idle,
                    # instead of polluting the collective window
                    if DBG and t == 0:
                        nc.sync.dma_start(out=dbg_yc_d[:], in_=ycur[:])
                    for qi, mg in enumerate((2 * t, 2 * t + 1)):
                        mt = mt_res[mg]
                        nc.vector.tensor_copy(mt[0:1, 0:4], ycur[0:1, 0:4])
                        (nc.scalar if qi == 0 else nc.gpsimd).dma_start(
                            out=mt[:],
                            in_=maskT_d[:, mg * 8 * RP:(mg + 1) * 8 * RP],
                        )

            # ---- local straight-through gumbel one-hot --------------
            logl = sb.tile([128, NB, C], f32)
            nc.vector.tensor_tensor(
                logl[:].rearrange("p b c -> p (b c)"), yloc[:], gumloc[:],
                mybir.AluOpType.add,
            )
            rmax = sb.tile([128, NB], f32)
            nc.vector.tensor_reduce(
                rmax[:], logl[:], axis=mybir.AxisListType.X,
                op=mybir.AluOpType.max,
            )
            yh16 = sb.tile([128, NB * C], f16)
            nc.vector.tensor_tensor(
                yh16[:].rearrange("p (b c) -> p b c", c=C),
                logl[:],
                rmax[:].unsqueeze(2).broadcast_to([128, NB, C]),
                mybir.AluOpType.is_equal,
            )
            nc.vector.copy_predicated(yh16[:], mloc[:], lloc[:])
            if DBG:
                nc.sync.dma_start(out=dbg_yh_d[:], in_=yh16[:])
            nc.vector.tensor_copy(yloc8[:], yh16[:])

            # last two mask groups reuse adjT pool slots: adjT's final
            # reads happen in pass 9, so the slot dependency itself defers
            # these loads to exactly when the space frees up
            for qi, g in enumerate(range(MRES, NG)):
                mt = sb.tile([128, 8 * RP], f8, name=f"mts{g}", tag=f"at{g}")
                (nc.scalar if qi == 0 else nc.gpsimd).dma_start(
                    out=mt[:], in_=maskT_d[:, g * 8 * RP:(g + 1) * 8 * RP],
                )
                mt_s.append(mt)

            cc8_in = dram.tile([128, NB * C], f8, tag="ccin")
            cc8_out = dram.tile([NCORES * 128, NB * C], f8, tag="ccout",
                                addr_space="Shared")
            nc.sync.dma_start(out=cc8_in[:], in_=yloc8[:])
            nc.gpsimd.collective_compute(
                "AllGather", mybir.AluOpType.bypass,
                replica_groups=[list(range(NCORES))],
                ins=[cc8_in[:]], outs=[cc8_out[:]],
            )
            nc.sync.dma_start(
                out=ycur[:].rearrange("p (g x) -> p g x", g=NCORES),
                in_=cc8_out[:].rearrange("(g p) x -> p g x", p=128),
            )

            # ---- final pass: dist^T = Yh^T @ maskT ------------------
            dacc = ps.tile([128, 384], f32, tag="acc")
            mm_pass(dacc, ycur,
                    lambda g: mt_res[g] if g < MRES else mt_s[g - MRES])

            # ---- normalize + squared error --------------------------
            dT = sb.tile([128, 384], f32)
            strip_copies(dT, dacc)
            trd = ps.tile([128, NB * C], f32, tag="trd")
            transposes(trd, dT, id32)
            dist = sb.tile([128, NB, C], f32)
            nc.vector.tensor_copy(dist[:].rearrange("p b c -> p (b c)"), trd[:])
            rsum = sb.tile([128, NB], f32)
            nc.vector.tensor_reduce(
                rsum[:], dist[:], axis=mybir.AxisListType.X,
                op=mybir.AluOpType.add,
            )
            # valid rows always have rsum >= 1 (self-loop); clamp the
            # zero pad rows so 1/rsum stays finite (their dist is 0)
            nc.vector.tensor_scalar_max(rsum[:], rsum[:], 0.5)
            rinv = sb.tile([128, NB], f32)
            nc.vector.reciprocal(rinv[:], rsum[:])
            dd = sb.tile([128, NB, C], f32)
            nc.vector.tensor_tensor(
                dd[:], dist[:],
                rinv[:].unsqueeze(2).broadcast_to([128, NB, C]),
                mybir.AluOpType.mult,
            )
            if DBG:
                nc.sync.dma_start(out=dbg_dist_d[:],
                                  in_=dd[:].rearrange("p b c -> p (b c)"))
            nc.vector.tensor_tensor(
                dd[:].rearrange("p b c -> p (b c)"),
                dd[:].rearrange("p b c -> p (b c)"), pst[:],
                mybir.AluOpType.subtract,
            )
            nc.vector.tensor_tensor(
                dd[:], dd[:], dd[:], mybir.AluOpType.mult,
            )
            osq = sb.tile([128, NB], f32)
            nc.vector.tensor_reduce(
                osq[:], dd[:], axis=mybir.AxisListType.X,
                op=mybir.AluOpType.add,
            )
            nc.sync.dma_start(out=out_d[:], in_=osq[:])

    nc.compile()
    return nc


_nc = {}


def _get_program(tcch=12):
    if tcch not in _nc:
        _install_neff_cache()
        _nc[tcch] = build_program(tcch)
    return _nc[tcch]


def _tile_local(x, dtype):
    """[1250, cols] local slice -> [128, 10*cols] chunk-tiled, padded."""
    cols = x.shape[1]
    p = np.zeros((RP, cols), np.float32)
    p[:R] = x
    return np.ascontiguousarray(
        p.reshape(NB, 128, cols).transpose(1, 0, 2).reshape(128, NB * cols)
    ).astype(dtype)


def prep_inputs(adj, labels_onehot, pseudo_labels, gumbel, train_mask):
    import scipy.sparse as sp

    adj = np.asarray(adj, np.float32)
    labels = np.asarray(labels_onehot, np.float32)
    pseudo = np.asarray(pseudo_labels, np.float32)
    gumbel = np.asarray(gumbel, np.float32)
    m = np.asarray(train_mask).astype(bool)

    labm = labels * m[:, None]
    # fused 2-step operator: B = diag(1-m) adj (sparse ~30 nnz/row),
    # B2 = B @ B (~810 nnz/row), c2 = B c + c
    Bs = sp.csr_matrix(adj)
    keep = np.repeat(~m, np.diff(Bs.indptr))
    Bs.data = np.where(keep, Bs.data, 0.0).astype(np.float32)
    B2 = np.asarray((Bs @ Bs).todense(), np.float32)
    B2 *= SCALE
    c2 = np.asarray(Bs @ labm + labm, np.float32)

    # compact pass-0: only the train columns of B^2 ever multiply a
    # nonzero of c, so pass 0 contracts tcch*128 packed train columns
    tc = np.flatnonzero(m)
    nt = len(tc)
    tcch = max(1, (nt + 127) // 128)
    ntp = tcch * 128
    y0cp = np.zeros((ntp, C), np.float32)
    y0cp[:nt] = labels[tc]
    y0c = np.ascontiguousarray(
        y0cp.reshape(tcch, 128, C).transpose(1, 0, 2).reshape(128, tcch * C)
    ).astype(F8)

    # transpose identities (the HW transpose path ignores non-1 values,
    # so the SCALE descale lives in the fp8 cast / prescaled inputs)
    id16 = np.zeros((128, C), np.float16)
    id32 = np.zeros((128, C), np.float32)
    for s in range(4):
        for i in range(C):
            id16[32 * s + i, i] = 1.0
            id32[32 * s + i, i] = 1.0

    def tileT(M):
        """[R, N] row-block -> padded p-major [128, NCH*RP]."""
        blk = np.ascontiguousarray(M.T)                     # [N, R]
        padT = np.zeros((NCORES, RP, RP), np.float32)
        padT[:, :R, :R] = blk.reshape(NCORES, R, R)
        return np.ascontiguousarray(
            padT.reshape(NCH, 128, RP).transpose(1, 0, 2).reshape(128, NCH * RP)
        )

    in_maps = []
    for c in range(NCORES):
        rows = slice(c * R, (c + 1) * R)
        adjT8 = tileT(B2[rows, :]).astype(F8)
        maskT8 = (tileT(adj[rows, :]) != 0).astype(F8)
        bct = np.zeros((ntp, RP), np.float32)
        bct[:nt, :R] = B2[rows, :][:, tc].T
        b2tc8 = np.ascontiguousarray(
            bct.reshape(tcch, 128, RP).transpose(1, 0, 2)
            .reshape(128, tcch * RP)).astype(F8)
        # on-chip Y state is carried at 64x scale (argmax of the gumbel
        # logits is scale-invariant, so gumbel is prescaled to match)
        gl = _tile_local(gumbel[rows] * SCALE, np.float32)
        c2l = _tile_local(c2[rows] * SCALE, np.float16)
        ll = _tile_local(labm[rows], np.float16)
        ml = _tile_local(np.repeat(m[rows, None].astype(np.float32), C, 1),
                         np.uint8)
        pl = _tile_local(pseudo[rows], np.float32)
        in_maps.append({
            "adjT8": adjT8, "maskT8": maskT8, "b2tc8": b2tc8, "y0c": y0c,
            "gumloc": gl, "c2loc": c2l, "lloc": ll, "mloc": ml, "pst": pl,
            "id416": id16, "id432": id32,
        })
    return in_maps, tcch


def run_on_device(in_maps, tcch=12, trace=False, **kw):
    nc = _get_program(tcch)
    return run_bass_kernel_spmd(nc, in_maps, list(range(NCORES)), trace=trace, **kw)


def kernel(adj, labels_onehot, pseudo_labels, gumbel, train_mask,
           iter_step=10, k_hop=1, **_unused):
    assert int(iter_step) == 10 and int(k_hop) == 1, "kernel hardcodes 10/1"
    in_maps, tcch = prep_inputs(adj, labels_onehot, pseudo_labels, gumbel,
                                train_mask)
    res = run_on_device(in_maps, tcch)
    total = 0.0
    for c in range(NCORES):
        sq = np.asarray(res.results[c]["out_sq"], np.float64)
        total += sq.sum()
    return np.float32(total / (N * C))



# revision 57
# speedup vs baseline: 1.1665x; 1.1665x over previous
"""Trainium2 Bass kernel for nn_ContextLabel (GNN label propagation).

Computation: 10 iterations of Y = masked(adj @ Y) on [10000,16], then
straight-through gumbel one-hot, dist = (adj!=0) @ Yh row-normalized,
output mean((dist - pseudo_labels)^2)  (scalar).

The per-step update is affine: Y <- B @ Y + c with B = diag(1-m) adj
and c = m*labels, so two steps fuse into one pass with the squared
operator: Y <- B^2 @ Y + (Bc + c).  The device runs 5 fused passes
(temporal blocking) instead of 10 - identical HBM traffic (one operator
matrix + one mask matrix streamed per core, both fp8), but half the
tensor-engine passes and half the AllGathers.  B^2 is ~810 nnz/row and
is scaled by 64 before fp8 quantization (entries ~1e-3 would underflow
e4m3 otherwise); the on-chip Y state runs at 64x scale (c2 and gumbel
prescaled on host, argmax is scale-invariant) and the exact 1/64
descale rides the fp8 cast feeding each AllGather, so it costs nothing.

Strategy (8 NeuronCores, row-parallel, padded to 1280 rows/core):
 - core c owns rows [1250c, 1250c+1250), zero-padded to 1280 so the
   padded N is 10240 = 80 chunks of 128 exactly.
 - (64*B^2)^T shard (fp8 e4m3, [10240 x 1280]) stays RESIDENT in SBUF;
   all 5 passes stream it from SBUF through the tensor engine with
   Y stationary: out^T[16,1280] = Y^T @ opT, split over 4 PE column
   groups (tile_position at partitions 0/32/64/96) so the four matmuls
   of each contraction chunk stream concurrently (~3x PE throughput).
 - per-iteration AllGather of the fp8 Y slice in chunk-tiled p-major
   layout [128,10,16] so both collective-side DMAs are clean line
   transfers; operator loads are spread over all 3 DMA-capable queues
   (scalar/gpsimd/sync, ~100GB/s each) and mask prefetches are
   dependency-gated into pass windows (2 per window, 2 queues) so
   nothing contends with the latency-bound collectives.
 - COMPACT PASS 0: the initial Y is c = mask*labels, zero outside the
   ~1000 train rows, so pass 0 contracts only the packed train columns
   of B^2 (exact, ~1.9MB vs 12.5MB).  Every core reaches the first
   AllGather ~8us after ITS start instead of after the full operator
   load, so cross-core startup skew overlaps the background load of
   the full operator (used from pass 1 on).
 - gumbel straight-through one-hot computed LOCALLY on the core's own
   rows; the final exchange gathers the fp8 one-hot (exact in fp8).
 - final pass streams the 0/1 mask of the ORIGINAL adj (fp8): 8/10
   groups prefetched into SBUF during iteration pass windows, the last
   2 loaded into the freed operator pool slots during pass 5.
Verified on host: 1 argmax flip out of 10000 rows, final relerr ~4e-6
(tolerance 2e-2).
"""

import hashlib
import os
import shutil
import sys
from pathlib import Path

import numpy as np
import ml_dtypes

sys.path.insert(0, "/opt/trn_rl_repo")

import concourse.bass as bass  # noqa: E402
import concourse.mybir as mybir  # noqa: E402
import concourse.tile as tile  # noqa: E402
from concourse import bacc  # noqa: E402
import concourse.bass2jax as bass2jax  # noqa: E402
from concourse.bass_utils import run_bass_kernel_spmd  # noqa: E402

F8 = ml_dtypes.float8_e4m3
NCORES = 8
N = 10000
C = 16
R = N // NCORES           # 1250 real rows per core
RP = 1280                 # padded rows per core
NP = RP * NCORES          # 10240 padded N
NB = RP // 128            # 10 local blocks of 128 rows
NCH = NP // 128           # 80 contraction chunks of 128
NG = NCH // 8             # 10 adjT groups of 8 chunks
# PE column-group strips: (partition base, col offset, width)
STRIPS = [(0, 0, 384), (32, 384, 384), (64, 768, 256), (96, 1024, 256)]
# local block b -> (strip partition base, col offset within strip)
BLK = [(0, 0), (0, 128), (0, 256), (32, 0), (32, 128), (32, 256),
       (64, 0), (64, 128), (96, 0), (96, 128)]
MRES = 8                  # mask groups resident in SBUF
NITER = 5                 # fused 2-step passes (10 reference steps)
SCALE = 64.0              # fp8 scale for B^2 (descaled in the AG cast)
DR = False                # fp8 DoubleRow matmuls (K=256 per instruction)

_NEFF_CACHE = Path.home() / ".cache" / "bass_neff"


def _install_neff_cache():
    orig = bass2jax.compile_bir_kernel
    if getattr(bass2jax.compile_bir_kernel, "_cached", False):
        return

    def cached(bir_json, tmpdir, neff_name="file.neff"):
        h = hashlib.sha256(bir_json).hexdigest()
        p = _NEFF_CACHE / f"{h}.neff"
        dst = os.path.join(tmpdir, neff_name)
        if p.exists():
            shutil.copy(p, dst)
            return dst
        out = orig(bir_json, tmpdir, neff_name)
        try:
            _NEFF_CACHE.mkdir(parents=True, exist_ok=True)
            shutil.copy(out, p)
        except OSError:
            pass
        return out

    cached._cached = True
    bass2jax.compile_bir_kernel = cached


def build_program(tcch=12):
    nc = bacc.Bacc(
        "TRN2", target_bir_lowering=False, debug=False,
        enable_asserts=False, num_devices=NCORES,
    )
    f8, f16, f32 = mybir.dt.float8e4, mybir.dt.float16, mybir.dt.float32
    u8 = mybir.dt.uint8

    # pre-tiled p-major [128, chunk, col] so group loads are contiguous
    adjT_d = nc.dram_tensor("adjT8", [128, NCH * RP], f8, kind="ExternalInput")
    maskT_d = nc.dram_tensor("maskT8", [128, NCH * RP], f8, kind="ExternalInput")
    # compact pass-0 operand: B^2 restricted to the train columns
    # (c is zero elsewhere, so this is exact), tcch chunks of 128
    btc_d = nc.dram_tensor("b2tc8", [128, tcch * RP], f8,
                           kind="ExternalInput")
    y0c_d = nc.dram_tensor("y0c", [128, tcch * C], f8, kind="ExternalInput")
    guml_d = nc.dram_tensor("gumloc", [128, NB * C], f32, kind="ExternalInput")
    c2_d = nc.dram_tensor("c2loc", [128, NB * C], f16, kind="ExternalInput")
    lloc_d = nc.dram_tensor("lloc", [128, NB * C], f16, kind="ExternalInput")
    mloc_d = nc.dram_tensor("mloc", [128, NB * C], u8, kind="ExternalInput")
    pst_d = nc.dram_tensor("pst", [128, NB * C], f32, kind="ExternalInput")
    id16_d = nc.dram_tensor("id416", [128, C], f16, kind="ExternalInput")
    id32_d = nc.dram_tensor("id432", [128, C], f32, kind="ExternalInput")
    out_d = nc.dram_tensor("out_sq", [128, NB], f32, kind="ExternalOutput")
    DBG = False
    if DBG:
        dbg_y1_d = nc.dram_tensor("dbg_y1", [128, NB * C], f16,
                                  kind="ExternalOutput")
        dbg_yc_d = nc.dram_tensor("dbg_yc", [128, NCH * C], f8,
                                  kind="ExternalOutput")
        dbg_y5_d = nc.dram_tensor("dbg_y5", [128, NB * C], f16,
                                  kind="ExternalOutput")
        dbg_yh_d = nc.dram_tensor("dbg_yh", [128, NB * C], f16,
                                  kind="ExternalOutput")
        dbg_dist_d = nc.dram_tensor("dbg_dist", [128, NB * C], f32,
                                    kind="ExternalOutput")

    with tile.TileContext(nc) as tc:
        with (
            tc.tile_pool(name="sb", bufs=1) as sb,
            tc.tile_pool(name="ps", bufs=2, space="PSUM") as ps,
            tc.tile_pool(name="dram", bufs=2, space="DRAM") as dram,
        ):
            # ---- resident tiles -------------------------------------
            at_g = [sb.tile([128, 8 * RP], f8, name=f"at{g}", tag=f"at{g}")
                    for g in range(NG)]
            mt_res = [sb.tile([128, 8 * RP], f8, name=f"mt{g}", tag=f"mt{g}")
                      for g in range(MRES)]
            ycur = sb.tile([128, NCH * C], f8)
            btc = sb.tile([128, tcch * RP], f8)
            y0c = sb.tile([128, tcch * C], f8)
            yT = sb.tile([128, 384], f16)
            yloc = sb.tile([128, NB * C], f16)
            yloc8 = sb.tile([128, NB * C], f8)
            gumloc = sb.tile([128, NB * C], f32)
            c2loc = sb.tile([128, NB * C], f16)
            lloc = sb.tile([128, NB * C], f16)
            mloc = sb.tile([128, NB * C], u8)
            pst = sb.tile([128, NB * C], f32)
            id16 = sb.tile([128, C], f16)
            id32 = sb.tile([128, C], f32)

            # ---- initial loads --------------------------------------
            # small tensors first (sync queue); adjT group loads on two
            # queues, even/odd interleaved so arrival tracks consumption
            # compact pass-0 operands first (small, 3 queues) so every
            # core reaches the first AllGather fast regardless of skew
            nc.sync.dma_start(out=y0c[:], in_=y0c_d[:])
            third = (tcch + 2) // 3 * RP
            for qi, eng in enumerate((nc.sync, nc.scalar, nc.gpsimd)):
                lo, hi = qi * third, min((qi + 1) * third, tcch * RP)
                if lo < hi:
                    eng.dma_start(out=btc[:, lo:hi], in_=btc_d[:, lo:hi])
            nc.sync.dma_start(out=id16[:], in_=id16_d[:])
            nc.sync.dma_start(out=c2loc[:], in_=c2_d[:])
            nc.sync.dma_start(out=lloc[:], in_=lloc_d[:])
            nc.sync.dma_start(out=mloc[:], in_=mloc_d[:])
            nc.sync.dma_start(out=gumloc[:], in_=guml_d[:])
            nc.sync.dma_start(out=pst[:], in_=pst_d[:])
            nc.sync.dma_start(out=id32[:], in_=id32_d[:])
            # operator loads on 4 DMA queues (each sustains ~100GB/s; the
            # HBM roofline needs all of them), interleaved so arrival
            # order tracks pass-0 consumption
            load_engs = [nc.scalar, nc.gpsimd, nc.sync]
            for g in range(NG):
                load_engs[g % 3].dma_start(
                    out=at_g[g][:],
                    in_=adjT_d[:, g * 8 * RP:(g + 1) * 8 * RP],
                )

            def mm_pass0(acc):
                """compact pass 0: contract only the train columns."""
                for k in range(tcch):
                    lhsT = y0c[:, k * C:(k + 1) * C]
                    for (pb, co, w) in STRIPS:
                        nc.tensor.matmul(
                            acc[pb:pb + C, 0:w],
                            lhsT,
                            btc[:, k * RP + co:k * RP + co + w],
                            start=(k == 0), stop=(k == tcch - 1),
                            tile_position=(0, pb),
                        )

            def mm_pass(acc, lhs_tile, chunk_tile):
                """acc strips += lhs.T @ adjT over all 80 chunks.

                Chunk-major so the 4 column-group matmuls of each chunk
                stream concurrently through the PE array.
                """
                if DR:
                    for k in range(0, NCH, 2):
                        g, j = divmod(k, 8)
                        lhsT = lhs_tile[:, k * C:(k + 2) * C].rearrange(
                            "p (two c) -> p c two", two=2)
                        rt3 = chunk_tile(g)[:].rearrange(
                            "p (j x) -> p x j", j=8)
                        for (pb, co, w) in STRIPS:
                            nc.tensor.matmul(
                                acc[pb:pb + C, 0:w],
                                lhsT,
                                rt3[:, co:co + w, j:j + 2],
                                start=(k == 0), stop=(k == NCH - 2),
                                tile_position=(0, pb),
                                perf_mode=mybir.MatmulPerfMode.DoubleRow,
                            )
                    return
                for k in range(NCH):
                    g, j = divmod(k, 8)
                    lhsT = lhs_tile[:, k * C:(k + 1) * C]
                    rt = chunk_tile(g)
                    for (pb, co, w) in STRIPS:
                        nc.tensor.matmul(
                            acc[pb:pb + C, 0:w],
                            lhsT,
                            rt[:, j * RP + co:j * RP + co + w],
                            start=(k == 0), stop=(k == NCH - 1),
                            tile_position=(0, pb),
                        )

            def strip_copies(dst, acc):
                """psum strips -> sbuf, alternating scalar/vector engines."""
                for i, (pb, co, w) in enumerate(STRIPS):
                    if i % 2 == 0:
                        nc.scalar.copy(dst[pb:pb + C, 0:w], acc[pb:pb + C, 0:w])
                    else:
                        nc.vector.tensor_copy(dst[pb:pb + C, 0:w],
                                              acc[pb:pb + C, 0:w])

            def transposes(trp, src, ident):
                for b in range(NB):
                    pb, o = BLK[b]
                    nc.tensor.transpose(
                        trp[:, b * C:(b + 1) * C],
                        src[pb:pb + C, o:o + 128],
                        ident[pb:pb + C, :],
                        tile_position=(pb, 0),
                    )

            # ---- 5 fused 2-step propagation passes ------------------
            mt_s = []
            for t in range(NITER):
                acc = ps.tile([128, 384], f32, name=f"acc{t}", tag="acc")
                if t == 0:
                    mm_pass0(acc)
                else:
                    mm_pass(acc, ycur, lambda g: at_g[g])
                strip_copies(yT, acc)
                trp = ps.tile([128, NB * C], f16, name=f"trp{t}", tag="trp")
                transposes(trp, yT, id16)
                # 64Y <- (64 B^2) Y + 64(Bc + c)
                nc.vector.tensor_tensor(yloc[:], trp[:], c2loc[:],
                                        mybir.AluOpType.add)
                if DBG and t == 0:
                    nc.sync.dma_start(out=dbg_y1_d[:], in_=yloc[:])
                if DBG and t == NITER - 1:
                    nc.sync.dma_start(out=dbg_y5_d[:], in_=yloc[:])
                if t < NITER - 1:
                    # yloc carries 64*Y; descale exactly in the fp8 cast
                    nc.vector.tensor_scalar_mul(yloc8[:], yloc[:], 1.0 / SCALE)
                    cc_in = dram.tile([128, NB * C], f8, name=f"ccin{t}",
                                      tag="ccin")
                    cc_out = dram.tile([NCORES * 128, NB * C], f8,
                                       name=f"ccout{t}", tag="ccout",
                                       addr_space="Shared")
                    nc.sync.dma_start(out=cc_in[:], in_=yloc8[:])
                    nc.gpsimd.collective_compute(
                        "AllGather", mybir.AluOpType.bypass,
                        replica_groups=[list(range(NCORES))],
                        ins=[cc_in[:]], outs=[cc_out[:]],
                    )
                    nc.sync.dma_start(
                        out=ycur[:].rearrange("p (g x) -> p g x", g=NCORES),
                        in_=cc_out[:].rearrange("(g p) x -> p g x", p=128),
                    )
                    # mask prefetch, gated on the post-collective gather
                    # (tiny write into the target tile) so the load runs
                    # during the NEXT pass, when HBM is otherwise idle,
                    # instead of polluting the collective window
                    if DBG and t == 0:
                        nc.sync.dma_start(out=dbg_yc_d[:], in_=ycur[:])
                    for qi, mg in enumerate((2 * t, 2 * t + 1)):
                        mt = mt_res[mg]
                        nc.vector.tensor_copy(mt[0:1, 0:4], ycur[0:1, 0:4])
                        (nc.scalar if qi == 0 else nc.gpsimd).dma_start(
                            out=mt[:],
                            in_=maskT_d[:, mg * 8 * RP:(mg + 1) * 8 * RP],
                        )

            # ---- local straight-through gumbel one-hot --------------
            logl = sb.tile([128, NB, C], f32)
            nc.vector.tensor_tensor(
                logl[:].rearrange("p b c -> p (b c)"), yloc[:], gumloc[:],
                mybir.AluOpType.add,
            )
            rmax = sb.tile([128, NB], f32)
            nc.vector.tensor_reduce(
                rmax[:], logl[:], axis=mybir.AxisListType.X,
                op=mybir.AluOpType.max,
            )
            yh16 = sb.tile([128, NB * C], f16)
            nc.vector.tensor_tensor(
                yh16[:].rearrange("p (b c) -> p b c", c=C),
                logl[:],
                rmax[:].unsqueeze(2).broadcast_to([128, NB, C]),
                mybir.AluOpType.is_equal,
            )
            nc.vector.copy_predicated(yh16[:], mloc[:], lloc[:])
            if DBG:
                nc.sync.dma_start(out=dbg_yh_d[:], in_=yh16[:])
            nc.vector.tensor_copy(yloc8[:], yh16[:])

            # last two mask groups reuse adjT pool slots: adjT's final
            # reads happen in pass 9, so the slot dependency itself defers
            # these loads to exactly when the space frees up
            for qi, g in enumerate(range(MRES, NG)):
                mt = sb.tile([128, 8 * RP], f8, name=f"mts{g}", tag=f"at{g}")
                (nc.scalar if qi == 0 else nc.gpsimd).dma_start(
                    out=mt[:], in_=maskT_d[:, g * 8 * RP:(g + 1) * 8 * RP],
                )
                mt_s.append(mt)

            cc8_in = dram.tile([128, NB * C], f8, tag="ccin")
            cc8_out = dram.tile([NCORES * 128, NB * C], f8, tag="ccout",
                                addr_space="Shared")
            nc.sync.dma_start(out=cc8_in[:], in_=yloc8[:])
            nc.gpsimd.collective_compute(
                "AllGather", mybir.AluOpType.bypass,
                replica_groups=[list(range(NCORES))],
                ins=[cc8_in[:]], outs=[cc8_out[:]],
            )
            nc.sync.dma_start(
                out=ycur[:].rearrange("p (g x) -> p g x", g=NCORES),
                in_=cc8_out[:].rearrange("(g p) x -> p g x", p=128),
            )

            # ---- final pass: dist^T = Yh^T @ maskT ------------------
            dacc = ps.tile([128, 384], f32, tag="acc")
            mm_pass(dacc, ycur,
                    lambda g: mt_res[g] if g < MRES else mt_s[g - MRES])

            # ---- normalize + squared error --------------------------
            dT = sb.tile([128, 384], f32)
            strip_copies(dT, dacc)
            trd = ps.tile([128, NB * C], f32, tag="trd")
            transposes(trd, dT, id32)
            dist = sb.tile([128, NB, C], f32)
            nc.vector.tensor_copy(dist[:].rearrange("p b c -> p (b c)"), trd[:])
            rsum = sb.tile([128, NB], f32)
            nc.vector.tensor_reduce(
                rsum[:], dist[:], axis=mybir.AxisListType.X,
                op=mybir.AluOpType.add,
            )
            # valid rows always have rsum >= 1 (self-loop); clamp the
            # zero pad rows so 1/rsum stays finite (their dist is 0)
            nc.vector.tensor_scalar_max(rsum[:], rsum[:], 0.5)
            rinv = sb.tile([128, NB], f32)
            nc.vector.reciprocal(rinv[:], rsum[:])
            dd = sb.tile([128, NB, C], f32)
            nc.vector.tensor_tensor(
                dd[:], dist[:],
                rinv[:].unsqueeze(2).broadcast_to([128, NB, C]),
                mybir.AluOpType.mult,
            )
            if DBG:
                nc.sync.dma_start(out=dbg_dist_d[:],
                                  in_=dd[:].rearrange("p b c -> p (b c)"))
            nc.vector.tensor_tensor(
                dd[:].rearrange("p b c -> p (b c)"),
                dd[:].rearrange("p b c -> p (b c)"), pst[:],
                mybir.AluOpType.subtract,
            )
            nc.vector.tensor_tensor(
                dd[:], dd[:], dd[:], mybir.AluOpType.mult,
            )
            osq = sb.tile([128, NB], f32)
            nc.vector.tensor_reduce(
                osq[:], dd[:], axis=mybir.AxisListType.X,
                op=mybir.AluOpType.add,
            )
            nc.sync.dma_start(out=out_d[:], in_=osq[:])

    nc.compile()
    return nc


_nc = {}


def _get_program(tcch=12):
    if tcch not in _nc:
        _install_neff_cache()
        _nc[tcch] = build_program(tcch)
    return _nc[tcch]


def _tile_local(x, dtype):
    """[1250, cols] local slice -> [128, 10*cols] chunk-tiled, padded."""
    cols = x.shape[1]
    p = np.zeros((RP, cols), np.float32)
    p[:R] = x
    return np.ascontiguousarray(
        p.reshape(NB, 128, cols).transpose(1, 0, 2).reshape(128, NB * cols)
    ).astype(dtype)


def prep_inputs(adj, labels_onehot, pseudo_labels, gumbel, train_mask):
    import scipy.sparse as sp

    adj = np.asarray(adj, np.float32)
    labels = np.asarray(labels_onehot, np.float32)
    pseudo = np.asarray(pseudo_labels, np.float32)
    gumbel = np.asarray(gumbel, np.float32)
    m = np.asarray(train_mask).astype(bool)

    labm = labels * m[:, None]
    # fused 2-step operator: B = diag(1-m) adj (sparse ~30 nnz/row),
    # B2 = B @ B (~810 nnz/row), c2 = B c + c
    Bs = sp.csr_matrix(adj)
    keep = np.repeat(~m, np.diff(Bs.indptr))
    Bs.data = np.where(keep, Bs.data, 0.0).astype(np.float32)
    B2 = np.asarray((Bs @ Bs).todense(), np.float32)
    B2 *= SCALE
    c2 = np.asarray(Bs @ labm + labm, np.float32)

    # compact pass-0: only the train columns of B^2 ever multiply a
    # nonzero of c, so pass 0 contracts tcch*128 packed train columns
    tc = np.flatnonzero(m)
    nt = len(tc)
    tcch = max(1, (nt + 127) // 128)
    ntp = tcch * 128
    y0cp = np.zeros((ntp, C), np.float32)
    y0cp[:nt] = labels[tc]
    y0c = np.ascontiguousarray(
        y0cp.reshape(tcch, 128, C).transpose(1, 0, 2).reshape(128, tcch * C)
    ).astype(F8)

    # transpose identities (the HW transpose path ignores non-1 values,
    # so the SCALE descale lives in the fp8 cast / prescaled inputs)
    id16 = np.zeros((128, C), np.float16)
    id32 = np.zeros((128, C), np.float32)
    for s in range(4):
        for i in range(C):
            id16[32 * s + i, i] = 1.0
            id32[32 * s + i, i] = 1.0

    def tileT(M):
        """[R, N] row-block -> padded p-major [128, NCH*RP]."""
        blk = np.ascontiguousarray(M.T)                     # [N, R]
        padT = np.zeros((NCORES, RP, RP), np.float32)
        padT[:, :R, :R] = blk.reshape(NCORES, R, R)
        return np.ascontiguousarray(
            padT.reshape(NCH, 128, RP).transpose(1, 0, 2).reshape(128, NCH * RP)
        )

    in_maps = []
    for c in range(NCORES):
        rows = slice(c * R, (c + 1) * R)
        adjT8 = tileT(B2[rows, :]).astype(F8)
        maskT8 = (tileT(adj[rows, :]) != 0).astype(F8)
        bct = np.zeros((ntp, RP), np.float32)
        bct[:nt, :R] = B2[rows, :][:, tc].T
        b2tc8 = np.ascontiguousarray(
            bct.reshape(tcch, 128, RP).transpose(1, 0, 2)
            .reshape(128, tcch * RP)).astype(F8)
        # on-chip Y state is carried at 64x scale (argmax of the gumbel
        # logits is scale-invariant, so gumbel is prescaled to match)
        gl = _tile_local(gumbel[rows] * SCALE, np.float32)
        c2l = _tile_local(c2[rows] * SCALE, np.float16)
        ll = _tile_local(labm[rows], np.float16)
        ml = _tile_local(np.repeat(m[rows, None].astype(np.float32), C, 1),
                         np.uint8)
        pl = _tile_local(pseudo[rows], np.float32)
        in_maps.append({
            "adjT8": adjT8, "maskT8": maskT8, "b2tc8": b2tc8, "y0c": y0c,
            "gumloc": gl, "c2loc": c2l, "lloc": ll, "mloc": ml, "pst": pl,
            "id416": id16, "id432": id32,
        })
    return in_maps, tcch


def run_on_device(in_maps, tcch=12, trace=False, **kw):
    nc = _get_program(tcch)
    return run_bass_kernel_spmd(nc, in_maps, list(range(NCORES)), trace=trace, **kw)


def kernel(adj, labels_onehot, pseudo_labels, gumbel, train_mask,
           iter_step=10, k_hop=1, **_unused):
    assert int(iter_step) == 10 and int(k_hop) == 1, "kernel hardcodes 10/1"
    in_maps, tcch = prep_inputs(adj, labels_onehot, pseudo_labels, gumbel,
                                train_mask)
    res = run_on_device(in_maps, tcch)
    total = 0.0
    for c in range(NCORES):
        sq = np.asarray(res.results[c]["out_sq"], np.float64)
        total += sq.sum()
    return np.float32(total / (N * C))

